# revision 2
# baseline (speedup 1.0000x reference)
"""Trainium2 Bass kernel for DeepKernelNN GNN message passing (NNConv-style).

Strategy (8 NeuronCores, SPMD):
  - Host: sort edges by dst, shard contiguous 512-node dst ranges per core,
    pad each core to a common edge count. Precompute h0 = x@fc1+b (tiny),
    per-edge metadata (src idx, local dst, 1/deg), and augmented weights.
  - Device per layer: edge MLP feature-major (weights stationary on PE),
    We = e2@kw3 edge-major in bf16 into PSUM, staged to SBUF by the Act
    engine, per-edge matvec msg = h[src] . We on DVE with 8 independent
    per-partition-scalar FMA accumulators, segment-sum via one-hot scatter
    matmul (S built on device from iota/is_equal, inv_deg folded in; the
    scatter matmul is software-pipelined one tile behind the We matmuls so
    the PE never waits on the DVE), NNConv update feature-major, AllGather
    h across the 8 cores per layer.
  - Layer k+1 weights (kw2/kw3 in bf16) are double-buffered and prefetched
    on the gpsimd DMA queue while layer k computes.
"""

import sys

sys.path.insert(0, "/opt/trn_rl_repo")

import numpy as np
import ml_dtypes

import concourse.bass as bass
import concourse.mybir as mybir
import concourse.tile as tile_mod
from concourse.bass_utils import run_bass_kernel_spmd
from concourse.masks import make_identity
from concourse.tile import TileContext
from concourse.vector_clock import ScopedClock, VectorClock

# ----------------------------------------------------------------------------
# Toolchain workarounds: this walrus build rejects instructions carrying more
# than a couple of sync waits ("Too many sync wait commands").  Split waits
# onto dedicated same-engine NoOps.
# ----------------------------------------------------------------------------
WAIT_LIMIT = 1


def _patched_drain_and_barrier(self, tick_clock, wait_clock):
    nc = self.nc
    gc = tick_clock.global_clock
    n = len(gc)
    for i in range(n):
        t = gc[i]
        if t > 0:
            sub = [0] * n
            sub[i] = t
            nop_inst = nc.sync.nop(nofuse=True)
            wait_clock.add_sem_waits(nop_inst.ins, ScopedClock({None: VectorClock(sub)}))
    nc.sync.drain()
    nc.all_engine_barrier()
    popped = nc._tile_sem_poison_stack.pop()
    assert popped is self._sem_poison
    nc.clear_and_free_semaphores(list(self.sems.allocated().values()))
    nc.all_engine_barrier()


tile_mod.TileContext._drain_and_barrier = _patched_drain_and_barrier


def _split_excess_waits(nc, limit=WAIT_LIMIT):
    n_split = 0
    for _bbname, bbb in nc.bb_map.items():
        bb = bbb.bb
        insts = list(bb.instructions)
        out = []
        for inst in insts:
            si = inst.sync_info
            if si is not None and si.on_wait is not None and len(si.on_wait) > limit:
                waits = list(si.on_wait)
                movable = [w for w in waits if w.wait_reg is None]
                fixed = [w for w in waits if w.wait_reg is not None]
                nkeep_mov = max(0, limit - len(fixed))
                keep = fixed + (movable[len(movable) - nkeep_mov:] if nkeep_mov else [])
                extra = movable[: len(movable) - nkeep_mov]
                while extra:
                    chunk, extra = extra[:limit], extra[limit:]
                    nop = mybir.InstNoOp(name=nc.get_next_instruction_name())
                    nop.engine = inst.engine
                    nop.sync_info = mybir.SyncInfo(on_wait=chunk, on_update=[])
                    nop.bass_nofuse = True
                    nc.register_instruction(nop, overwrite=True)
                    out.append(nop)
                    n_split += 1
                inst.sync_info = mybir.SyncInfo(
                    on_wait=keep, on_update=list(si.on_update or [])
                )
            out.append(inst)
        bb.instructions = out
    return n_split


import concourse.bass_utils as _bu

_orig_run_command = _bu.run_command


def _run_command_no_birverifier(cmd, **kw):
    cmd = [
        c.replace("birverifier,", "") if isinstance(c, str) else c for c in cmd
    ]
    return _orig_run_command(cmd, **kw)


_bu.run_command = _run_command_no_birverifier


def _round_f32r(x):
    """Host replica of the fp32r rounding (RNE, 11 mantissa bits kept).

    PE fp32r operands must contain rounded bits: feeding raw fp32 bits makes
    the PE fault (verified on HW), so anything DMA'd into an fp32r tile is
    pre-rounded here.
    """
    ai = np.ascontiguousarray(x, np.float32).view(np.uint32)
    drop = np.uint32(12)
    half = np.uint32(1 << 11)
    lsb = ((ai >> drop) & np.uint32(1)).astype(np.uint32)
    out = ((ai + (half - np.uint32(1)) + lsb) >> drop) << drop
    return out.view(np.float32)


# ----------------------------------------------------------------------------
# Problem constants (hardcoded from the model definition)
# ----------------------------------------------------------------------------
N_NODES = 4096
N_EDGES = 32768
WIDTH = 64
KER_W = 1024
DEPTH = 4
KER_IN = 6
IN_W = 6
NCORES = 8
NLOC = N_NODES // NCORES  # 512 nodes per core
P = 128

_dt = mybir.dt
F32 = _dt.float32
F32R = _dt.float32r
BF16 = _dt.bfloat16
I32 = _dt.int32
ALU = mybir.AluOpType
AF = mybir.ActivationFunctionType
NACCS = 8
KC3 = KER_W // P  # 8 contraction chunks for kw3
NC3 = WIDTH * WIDTH  # 4096 output cols


def _build_nc(T, kb3_nonzero):
    """Build the SPMD Bass program for T 128-edge tiles per core."""
    B = (T + 3) // 4  # blocks of 512 edges (last may be ragged)
    EP = B * 512
    nc = bass.Bass("TRN2", target_bir_lowering=False, debug=False, num_devices=NCORES)

    # ---- dram inputs (per-core in_maps supply the data) ----
    d_h0g = nc.dram_tensor("h0g", [N_NODES, WIDTH], F32, kind="ExternalInput")
    d_hfm0 = nc.dram_tensor("hfm0", [WIDTH, NLOC], F32, kind="ExternalInput")
    d_kw1 = nc.dram_tensor("kw1a", [DEPTH, IN_W + 1, KER_W // 2], F32, kind="ExternalInput")
    d_kw2 = nc.dram_tensor("kw2b", [DEPTH, KER_W // 2, KER_W], BF16, kind="ExternalInput")
    d_kb2 = nc.dram_tensor("kb2s", [DEPTH, P, KER_W // P], F32, kind="ExternalInput")
    d_kw3 = nc.dram_tensor("kw3b", [DEPTH, KER_W, WIDTH * WIDTH], BF16, kind="ExternalInput")
    d_root = nc.dram_tensor("roota", [DEPTH, WIDTH + 1, WIDTH], F32, kind="ExternalInput")
    d_fc2 = nc.dram_tensor("fc2a", [WIDTH + 1, 1], F32, kind="ExternalInput")
    d_ea = nc.dram_tensor("eaT", [IN_W + 1, EP], F32, kind="ExternalInput")
    d_src = nc.dram_tensor("srci", [EP, 1], I32, kind="ExternalInput")
    d_dst = nc.dram_tensor("dstl", [EP, 1], F32, kind="ExternalInput")
    d_inv = nc.dram_tensor("invde", [EP, 1], F32, kind="ExternalInput")
    d_iota = nc.dram_tensor("iota", [P, NLOC], F32, kind="ExternalInput")
    d_kb3 = None
    if kb3_nonzero:
        d_kb3 = nc.dram_tensor("kb3r", [DEPTH, WIDTH, WIDTH], F32, kind="ExternalInput")

    d_out = nc.dram_tensor("out_loc", [1, NLOC], F32, kind="ExternalOutput")

    # ---- internal dram ----
    d_hloc = nc.dram_tensor("hloc", [NLOC, WIDTH], F32)
    d_hgat = [
        nc.dram_tensor(f"hgat{k}", [N_NODES, WIDTH], F32, addr_space="Shared")
        for k in range(DEPTH - 1)
    ]

    rg = [list(range(NCORES))]

    with TileContext(nc) as tc:
        with (
            tc.tile_pool(name="pers", bufs=1) as pers,
            tc.tile_pool(name="wk", bufs=2) as wk,
            tc.tile_pool(name="ppw", bufs=2, space="PSUM") as ppw,
            tc.tile_pool(name="ppe", bufs=2, space="PSUM") as ppe,
            tc.tile_pool(name="ppm", bufs=2, space="PSUM") as ppm,
            tc.tile_pool(name="ppa", bufs=1, space="PSUM") as ppa,
        ):
            # ---------------- persistent tiles ----------------
            iota_s = pers.tile([P, NLOC], F32)
            nc.sync.dma_start(out=iota_s[:], in_=d_iota[:])
            ident = pers.tile([P, P], F32)
            make_identity(nc, ident[:])

            # per-edge metadata as [128, 4B] (covers T used tiles)
            srcT = pers.tile([P, 4 * B], I32)
            dstT = pers.tile([P, 4 * B], F32)
            invT = pers.tile([P, 4 * B], F32)
            for (dsttile, dram) in ((srcT, d_src), (dstT, d_dst), (invT, d_inv)):
                nc.sync.dma_start(
                    out=dsttile[:],
                    in_=dram.ap().rearrange("(t p) o -> p (t o)", p=P),
                )

            # edge attrs, feature-major augmented [7, EP]; loaded once
            eaT = pers.tile([IN_W + 1, EP], F32R)
            nc.sync.dma_start(out=eaT[:].bitcast(F32), in_=d_ea[:])

            # h feature-major augmented [65, 512]; row 64 = ones
            # (hfm0 pre-rounded on host; device relu copies re-round later)
            hfm = pers.tile([WIDTH + 1, NLOC], F32R)
            nc.sync.dma_start(out=hfm[0:WIDTH, :].bitcast(F32), in_=d_hfm0[:])
            nc.vector.memset(hfm[WIDTH : WIDTH + 1, :].bitcast(F32), 1.0)

            fc2r = pers.tile([WIDTH + 1, 1], F32R)
            nc.sync.dma_start(out=fc2r[:].bitcast(F32), in_=d_fc2[:])

            # double-buffered per-layer weights
            kw1r = [pers.tile([IN_W + 1, KER_W // 2], F32R, name=f"kw1r{b}")
                    for b in range(2)]
            kw2rc = [
                [pers.tile([P, KER_W], BF16, name=f"kw2rc{b}_{c}")
                 for c in range(KER_W // 2 // P)]
                for b in range(2)
            ]
            kw3rc = [
                [pers.tile([P, NC3], BF16, name=f"kw3rc{b}_{c}") for c in range(KC3)]
                for b in range(2)
            ]
            rootr = [pers.tile([WIDTH + 1, WIDTH], F32R, name=f"rootr{b}")
                     for b in range(2)]
            kb2t = [pers.tile([P, KER_W // P], F32, name=f"kb2t{b}")
                    for b in range(2)]
            kb3t = [pers.tile([WIDTH, WIDTH], F32R, name=f"kb3t{b}")
                    for b in range(2)] if kb3_nonzero else None

            e1r = pers.tile([P, 4 * 512], BF16)
            e2r = pers.tile([P, 8 * 512], BF16)

            def load_weights(k, eng):
                bi = k % 2
                eng.dma_start(out=kw1r[bi][:].bitcast(F32), in_=d_kw1[k])
                for c in range(KER_W // 2 // P):
                    eng.dma_start(
                        out=kw2rc[bi][c][:], in_=d_kw2[k, c * P : (c + 1) * P, :]
                    )
                eng.dma_start(out=rootr[bi][:].bitcast(F32), in_=d_root[k])
                eng.dma_start(out=kb2t[bi][:], in_=d_kb2[k])
                if kb3_nonzero:
                    eng.dma_start(out=kb3t[bi][:].bitcast(F32), in_=d_kb3[k])
                for kc in range(KC3):
                    eng.dma_start(
                        out=kw3rc[bi][kc][:], in_=d_kw3[k, kc * P : (kc + 1) * P, :]
                    )

            load_weights(0, nc.sync)

            for k in range(DEPTH):
                bi = k % 2
                aggP = ppa.tile([WIDTH, NLOC], F32, tag="aggP")
                htab = d_h0g if k == 0 else d_hgat[k - 1]
                pend = [None]  # pending (msgr, S, t) awaiting scatter matmul

                def flush_scatter():
                    if pend[0] is None:
                        return
                    msgr_p, S_p, t_p = pend[0]
                    nc.tensor.matmul(
                        out=aggP[:], lhsT=msgr_p[:], rhs=S_p[:],
                        start=(t_p == 0), stop=False, skip_group_check=True,
                    )
                    pend[0] = None

                for blk in range(B):
                    eoff = blk * 512
                    # ---- e1 = relu(ea @ kw1_aug) : [512 feats, 512 edges] ----
                    for mc in range(4):
                        pe1 = ppe.tile([P, 512], F32, tag="pe")
                        nc.tensor.matmul(
                            out=pe1[:],
                            lhsT=kw1r[bi][:, mc * P : (mc + 1) * P],
                            rhs=eaT[:, eoff : eoff + 512],
                            start=True,
                            stop=True,
                        )
                        nc.scalar.activation(
                            e1r[:, mc * 512 : (mc + 1) * 512], pe1[:], AF.Relu
                        )
                    # prefetch next layer's weights once layer k is underway
                    if blk == 1 and k + 1 < DEPTH:
                        load_weights(k + 1, nc.gpsimd)
                    # ---- e2 = relu(e1 @ kw2 + kb2) : [1024 feats, 512 edges] ----
                    for mc2 in range(8):
                        pe2 = ppe.tile([P, 512], F32, tag="pe")
                        for kc in range(4):
                            nc.tensor.matmul(
                                out=pe2[:],
                                lhsT=kw2rc[bi][kc][:, mc2 * P : (mc2 + 1) * P],
                                rhs=e1r[:, kc * 512 : (kc + 1) * 512],
                                start=(kc == 0),
                                stop=(kc == 3),
                            )
                        nc.scalar.activation(
                            e2r[:, mc2 * 512 : (mc2 + 1) * 512],
                            pe2[:],
                            AF.Relu,
                            bias=kb2t[bi][:, mc2 : mc2 + 1],
                        )
                    # ---- per 128-edge tile (ragged last block) ----
                    for t4 in range(min(4, T - blk * 4)):
                        t = blk * 4 + t4
                        hsrc = wk.tile([P, WIDTH], F32, tag="hsrc")
                        nc.gpsimd.indirect_dma_start(
                            out=hsrc[:],
                            out_offset=None,
                            in_=htab[:],
                            in_offset=bass.IndirectOffsetOnAxis(
                                ap=srcT[:, t : t + 1], axis=0
                            ),
                        )
                        accs = [
                            wk.tile([P, WIDTH], F32, name=f"macc{j}_{t}",
                                    tag=f"macc{j}", bufs=2)
                            for j in range(NACCS)
                        ]
                        msgr = wk.tile([P, WIDTH], F32R, tag="msgr")
                        tcor = None
                        if kb3_nonzero:
                            tps = ppm.tile([WIDTH, P], F32, tag="tp")
                            nc.tensor.transpose(out=tps[:], in_=hsrc[:], identity=ident[:])
                            hsT = wk.tile([WIDTH, P], F32R, tag="hsT")
                            nc.scalar.activation(hsT[:], tps[:], AF.Copy)
                            tcor = ppm.tile([P, WIDTH], F32, tag="tc")
                            nc.tensor.matmul(
                                out=tcor[:], lhsT=hsT[:], rhs=kb3t[bi][:],
                                start=True, stop=True,
                            )
                        for cc in range(8):  # 512-col chunks of We
                            wps = ppw.tile([P, 512], F32, tag="wps")
                            for kc in range(KC3):
                                nc.tensor.matmul(
                                    out=wps[:],
                                    lhsT=e2r[:, kc * 512 + t4 * P : kc * 512 + (t4 + 1) * P],
                                    rhs=kw3rc[bi][kc][:, cc * 512 : (cc + 1) * 512],
                                    start=(kc == 0),
                                    stop=(kc == KC3 - 1),
                                )
                            # stage PSUM->SBUF on Act: DVE reads from SBUF are
                            # ~2x cheaper than PSUM reads
                            wsb = wk.tile([P, 512], F32, tag="wsb")
                            nc.scalar.activation(wsb[:], wps[:], AF.Copy)
                            if cc == 0:
                                # scatter for the previous tile: emitted after
                                # this tile's first We chunk so the PE never
                                # waits on the DVE matvec tail
                                flush_scatter()
                            for j in range(8):
                                i_ = cc * 8 + j
                                sl = wsb[:, j * WIDTH : (j + 1) * WIDTH]
                                sc = hsrc[:, i_ : i_ + 1]
                                ja = i_ % NACCS
                                if i_ < NACCS:
                                    nc.vector.tensor_scalar(
                                        out=accs[ja][:], in0=sl, scalar1=sc,
                                        scalar2=None, op0=ALU.mult,
                                    )
                                else:
                                    nc.vector.scalar_tensor_tensor(
                                        out=accs[ja][:], in0=sl, scalar=sc,
                                        in1=accs[ja][:], op0=ALU.mult, op1=ALU.add,
                                    )
                        # tree-reduce the 8 accumulators
                        for d in (0, 2, 4, 6):
                            nc.vector.tensor_add(
                                out=accs[d][:], in0=accs[d][:], in1=accs[d + 1][:]
                            )
                        nc.vector.tensor_add(out=accs[0][:], in0=accs[0][:], in1=accs[2][:])
                        if kb3_nonzero:
                            nc.vector.tensor_add(out=accs[4][:], in0=accs[4][:], in1=accs[6][:])
                            nc.vector.tensor_add(out=accs[0][:], in0=accs[0][:], in1=accs[4][:])
                            nc.vector.tensor_add(out=msgr[:], in0=accs[0][:], in1=tcor[:])
                        else:
                            nc.vector.tensor_add(out=accs[4][:], in0=accs[4][:], in1=accs[6][:])
                            nc.vector.tensor_add(out=msgr[:], in0=accs[0][:], in1=accs[4][:])
                        # ---- one-hot scatter weights (iota==dst)*invdeg ----
                        S = wk.tile([P, NLOC], F32R, tag="S")
                        nc.vector.tensor_scalar(
                            out=S[:], in0=iota_s[:], scalar1=dstT[:, t : t + 1],
                            scalar2=invT[:, t : t + 1], op0=ALU.is_equal, op1=ALU.mult,
                        )
                        pend[0] = (msgr, S, t)
                # ---- update: h = relu(agg*inv_deg(folded) + h@root + bias) ----
                flush_scatter()
                nc.tensor.matmul(
                    out=aggP[:], lhsT=rootr[bi][:], rhs=hfm[:],
                    start=False, stop=True, skip_group_check=True,
                )
                hnf = wk.tile([WIDTH, NLOC], F32, tag="hnf")
                nc.scalar.activation(hnf[:], aggP[:], AF.Relu)
                nc.scalar.activation(hfm[0:WIDTH, :], hnf[:], AF.Copy)
                if k < DEPTH - 1:
                    for c in range(NLOC // P):
                        tp = ppm.tile([P, WIDTH], F32, tag="tp")
                        nc.tensor.transpose(
                            out=tp[:],
                            in_=hnf[:, c * P : (c + 1) * P],
                            identity=ident[0:WIDTH, 0:WIDTH],
                        )
                        hts = wk.tile([P, WIDTH], F32, tag="hts")
                        nc.vector.tensor_copy(out=hts[:], in_=tp[:])
                        nc.sync.dma_start(out=d_hloc[c * P : (c + 1) * P, :], in_=hts[:])
                    nc.gpsimd.collective_compute(
                        "AllGather",
                        ALU.bypass,
                        ins=[d_hloc[:]],
                        outs=[d_hgat[k][:]],
                        replica_groups=rg,
                    )
            # ---- readout: out = h @ fc2 + b ----
            pf = ppm.tile([1, NLOC], F32, tag="tp")
            nc.tensor.matmul(out=pf[:], lhsT=fc2r[:], rhs=hfm[:], start=True, stop=True)
            ot = wk.tile([1, NLOC], F32, tag="hnf")
            nc.vector.tensor_copy(out=ot[:], in_=pf[:])
            nc.sync.dma_start(out=d_out[:], in_=ot[:])

    _split_excess_waits(nc)
    return nc


def _host_prep(x, edge_attr, fc1_w, fc1_b, kw1, kb1, kw2, kb2, kw3, kb3,
               root, bias, fc2_w, fc2_b, edge_index):
    f = np.float32
    bf = ml_dtypes.bfloat16
    x = np.asarray(x, f)
    edge_attr = np.asarray(edge_attr, f)
    fc1_w = np.asarray(fc1_w, f); fc1_b = np.asarray(fc1_b, f)
    kw1 = np.asarray(kw1, f); kb1 = np.asarray(kb1, f)
    kw2 = np.asarray(kw2, f); kb2 = np.asarray(kb2, f)
    kw3 = np.asarray(kw3, f); kb3 = np.asarray(kb3, f)
    root = np.asarray(root, f); bias = np.asarray(bias, f)
    fc2_w = np.asarray(fc2_w, f); fc2_b = np.asarray(fc2_b, f)
    ei = np.asarray(edge_index)
    src = ei[0].astype(np.int64)
    dst = ei[1].astype(np.int64)

    deg = np.bincount(dst, minlength=N_NODES).astype(f)
    inv_deg = np.zeros(N_NODES, f)
    np.divide(f(1.0), deg, out=inv_deg, where=deg > 0)

    order = np.argsort(dst, kind="stable")
    dsts = dst[order]
    bounds = np.searchsorted(dsts, np.arange(0, N_NODES + 1, NLOC))
    counts = np.diff(bounds)
    T = int(np.ceil(counts.max() / 128.0))
    EP = ((T + 3) // 4) * 512

    h0 = (x @ fc1_w + fc1_b).astype(f)

    kw1_aug = _round_f32r(np.concatenate([kw1, kb1[:, None, :]], axis=1))
    kw2b = kw2.astype(bf)
    kw3b = kw3.astype(bf)
    kb2s = np.stack([kb2[k].reshape(KER_W // P, P).T for k in range(DEPTH)]).astype(f)
    root_aug = _round_f32r(np.concatenate([root, bias[:, None, :]], axis=1))
    fc2_aug = _round_f32r(np.concatenate([fc2_w, fc2_b.reshape(1, 1)], axis=0))
    iota = np.tile(np.arange(NLOC, dtype=f)[None, :], (P, 1))
    kb3_nonzero = bool(np.any(kb3))
    kb3r = _round_f32r(kb3.reshape(DEPTH, WIDTH, WIDTH))

    in_maps = []
    for m in range(NCORES):
        sel = order[bounds[m] : bounds[m + 1]]
        cnt = len(sel)
        eaT = np.zeros((IN_W + 1, EP), f)
        eaT[0:IN_W, :cnt] = edge_attr[sel].T
        eaT[IN_W, :cnt] = 1.0
        eaT = _round_f32r(eaT)
        srci = np.zeros((EP, 1), np.int32)
        srci[:cnt, 0] = src[sel].astype(np.int32)
        dstl = np.full((EP, 1), -1.0, f)
        dstl[:cnt, 0] = (dst[sel] - NLOC * m).astype(f)
        invde = np.zeros((EP, 1), f)
        invde[:cnt, 0] = inv_deg[dst[sel]]
        im = {
            "h0g": h0,
            "hfm0": _round_f32r(np.ascontiguousarray(h0[NLOC * m : NLOC * (m + 1)].T)),
            "kw1a": kw1_aug,
            "kw2b": kw2b,
            "kb2s": kb2s,
            "kw3b": kw3b,
            "roota": root_aug,
            "fc2a": fc2_aug,
            "eaT": eaT,
            "srci": srci,
            "dstl": dstl,
            "invde": invde,
            "iota": iota,
        }
        if kb3_nonzero:
            im["kb3r"] = kb3r
        in_maps.append(im)
    return in_maps, T, kb3_nonzero


_BUILD_CACHE = {}


def kernel(**inputs) -> np.ndarray:
    in_maps, T, kb3_nonzero = _host_prep(**inputs)
    key = (T, kb3_nonzero)
    if key not in _BUILD_CACHE:
        _BUILD_CACHE[key] = _build_nc(T, kb3_nonzero)
    nc = _BUILD_CACHE[key]
    res = run_bass_kernel_spmd(nc, in_maps, list(range(NCORES)))
    out = np.concatenate(
        [res.results[m]["out_loc"].reshape(NLOC, 1) for m in range(NCORES)], axis=0
    )
    return out.astype(np.float32)


# revision 5
# speedup vs baseline: 1.0192x; 1.0192x over previous
"""Trainium2 Bass kernel for DeepKernelNN GNN message passing (NNConv-style).

Strategy (8 NeuronCores, SPMD):
  - Host: sort edges by dst, shard contiguous 512-node dst ranges per core,
    pad each core to a common edge count. Precompute h0 = x@fc1+b (tiny),
    per-edge metadata (src idx, local dst, 1/deg), and augmented weights.
  - Device per layer: edge MLP feature-major (weights stationary on PE),
    We = e2@kw3 edge-major in bf16 into PSUM, staged to SBUF by the Act
    engine, per-edge matvec msg = h[src] . We on DVE with 8 independent
    per-partition-scalar FMA accumulators, segment-sum via one-hot scatter
    matmul (S built on device from iota/is_equal, inv_deg folded in; the
    scatter matmul is software-pipelined one tile behind the We matmuls so
    the PE never waits on the DVE), NNConv update feature-major, AllGather
    h across the 8 cores per layer.
  - Layer k+1 weights (kw2/kw3 in bf16) are double-buffered and prefetched
    on the gpsimd DMA queue while layer k computes.
"""

import sys

sys.path.insert(0, "/opt/trn_rl_repo")

import numpy as np
import ml_dtypes

import concourse.bass as bass
import concourse.mybir as mybir
import concourse.tile as tile_mod
from concourse.bass_utils import run_bass_kernel_spmd
from concourse.masks import make_identity
from concourse.tile import TileContext
from concourse.vector_clock import ScopedClock, VectorClock

# ----------------------------------------------------------------------------
# Toolchain workarounds: this walrus build rejects instructions carrying more
# than a couple of sync waits ("Too many sync wait commands").  Split waits
# onto dedicated same-engine NoOps.
# ----------------------------------------------------------------------------
WAIT_LIMIT = 1


def _patched_drain_and_barrier(self, tick_clock, wait_clock):
    nc = self.nc
    gc = tick_clock.global_clock
    n = len(gc)
    for i in range(n):
        t = gc[i]
        if t > 0:
            sub = [0] * n
            sub[i] = t
            nop_inst = nc.sync.nop(nofuse=True)
            wait_clock.add_sem_waits(nop_inst.ins, ScopedClock({None: VectorClock(sub)}))
    nc.sync.drain()
    nc.all_engine_barrier()
    popped = nc._tile_sem_poison_stack.pop()
    assert popped is self._sem_poison
    nc.clear_and_free_semaphores(list(self.sems.allocated().values()))
    nc.all_engine_barrier()


tile_mod.TileContext._drain_and_barrier = _patched_drain_and_barrier


def _split_excess_waits(nc, limit=WAIT_LIMIT):
    n_split = 0
    for _bbname, bbb in nc.bb_map.items():
        bb = bbb.bb
        insts = list(bb.instructions)
        out = []
        for inst in insts:
            si = inst.sync_info
            if si is not None and si.on_wait is not None and len(si.on_wait) > limit:
                waits = list(si.on_wait)
                movable = [w for w in waits if w.wait_reg is None]
                fixed = [w for w in waits if w.wait_reg is not None]
                nkeep_mov = max(0, limit - len(fixed))
                keep = fixed + (movable[len(movable) - nkeep_mov:] if nkeep_mov else [])
                extra = movable[: len(movable) - nkeep_mov]
                while extra:
                    chunk, extra = extra[:limit], extra[limit:]
                    nop = mybir.InstNoOp(name=nc.get_next_instruction_name())
                    nop.engine = inst.engine
                    nop.sync_info = mybir.SyncInfo(on_wait=chunk, on_update=[])
                    nop.bass_nofuse = True
                    nc.register_instruction(nop, overwrite=True)
                    out.append(nop)
                    n_split += 1
                inst.sync_info = mybir.SyncInfo(
                    on_wait=keep, on_update=list(si.on_update or [])
                )
            out.append(inst)
        bb.instructions = out
    return n_split


import concourse.bass_utils as _bu

_orig_run_command = _bu.run_command


def _run_command_no_birverifier(cmd, **kw):
    cmd = [
        c.replace("birverifier,", "") if isinstance(c, str) else c for c in cmd
    ]
    return _orig_run_command(cmd, **kw)


_bu.run_command = _run_command_no_birverifier


def _round_f32r(x):
    """Host replica of the fp32r rounding (RNE, 11 mantissa bits kept).

    PE fp32r operands must contain rounded bits: feeding raw fp32 bits makes
    the PE fault (verified on HW), so anything DMA'd into an fp32r tile is
    pre-rounded here.
    """
    ai = np.ascontiguousarray(x, np.float32).view(np.uint32)
    drop = np.uint32(12)
    half = np.uint32(1 << 11)
    lsb = ((ai >> drop) & np.uint32(1)).astype(np.uint32)
    out = ((ai + (half - np.uint32(1)) + lsb) >> drop) << drop
    return out.view(np.float32)


# ----------------------------------------------------------------------------
# Problem constants (hardcoded from the model definition)
# ----------------------------------------------------------------------------
N_NODES = 4096
N_EDGES = 32768
WIDTH = 64
KER_W = 1024
DEPTH = 4
KER_IN = 6
IN_W = 6
NCORES = 8
NLOC = N_NODES // NCORES  # 512 nodes per core
P = 128

_dt = mybir.dt
F32 = _dt.float32
F32R = _dt.float32r
BF16 = _dt.bfloat16
I32 = _dt.int32
ALU = mybir.AluOpType
AF = mybir.ActivationFunctionType
NACCS = 8
KC3 = KER_W // P  # 8 contraction chunks for kw3
NC3 = WIDTH * WIDTH  # 4096 output cols


def _build_nc(T, kb3_nonzero):
    """Build the SPMD Bass program for T 128-edge tiles per core."""
    B = (T + 3) // 4  # blocks of 512 edges (last may be ragged)
    EP = B * 512
    nc = bass.Bass("TRN2", target_bir_lowering=False, debug=False, num_devices=NCORES)

    # ---- dram inputs (per-core in_maps supply the data) ----
    d_h0g = nc.dram_tensor("h0g", [N_NODES, WIDTH], BF16, kind="ExternalInput")
    d_hfm0 = nc.dram_tensor("hfm0", [WIDTH, NLOC], F32, kind="ExternalInput")
    d_kw1 = nc.dram_tensor("kw1a", [DEPTH, IN_W + 1, KER_W // 2], F32, kind="ExternalInput")
    d_kw2 = nc.dram_tensor("kw2b", [DEPTH, KER_W // 2, KER_W], BF16, kind="ExternalInput")
    d_kb2 = nc.dram_tensor("kb2s", [DEPTH, P, KER_W // P], F32, kind="ExternalInput")
    d_kw3 = nc.dram_tensor("kw3b", [DEPTH, KER_W, WIDTH * WIDTH], BF16, kind="ExternalInput")
    d_root = nc.dram_tensor("roota", [DEPTH, WIDTH + 1, WIDTH], F32, kind="ExternalInput")
    d_fc2 = nc.dram_tensor("fc2a", [WIDTH + 1, 1], F32, kind="ExternalInput")
    d_ea = nc.dram_tensor("eaT", [IN_W + 1, EP], F32, kind="ExternalInput")
    d_src = nc.dram_tensor("srci", [EP, 1], I32, kind="ExternalInput")
    d_dst = nc.dram_tensor("dstl", [EP, 1], F32, kind="ExternalInput")
    d_inv = nc.dram_tensor("invde", [EP, 1], F32, kind="ExternalInput")
    d_iota = nc.dram_tensor("iota", [P, NLOC], F32, kind="ExternalInput")
    d_kb3 = None
    if kb3_nonzero:
        d_kb3 = nc.dram_tensor("kb3r", [DEPTH, WIDTH, WIDTH], F32, kind="ExternalInput")

    d_out = nc.dram_tensor("out_loc", [1, NLOC], F32, kind="ExternalOutput")

    # ---- internal dram ----
    d_hloc = nc.dram_tensor("hloc", [NLOC, WIDTH], BF16)
    d_hgat = [
        nc.dram_tensor(f"hgat{k}", [N_NODES, WIDTH], BF16, addr_space="Shared")
        for k in range(DEPTH - 1)
    ]

    rg = [list(range(NCORES))]

    with TileContext(nc) as tc:
        with (
            tc.tile_pool(name="pers", bufs=1) as pers,
            tc.tile_pool(name="wk", bufs=2) as wk,
            tc.tile_pool(name="ppw", bufs=2, space="PSUM") as ppw,
            tc.tile_pool(name="ppe", bufs=2, space="PSUM") as ppe,
            tc.tile_pool(name="ppm", bufs=2, space="PSUM") as ppm,
            tc.tile_pool(name="ppa", bufs=1, space="PSUM") as ppa,
        ):
            # ---------------- persistent tiles ----------------
            iota_s = pers.tile([P, NLOC], F32)
            nc.sync.dma_start(out=iota_s[:], in_=d_iota[:])
            ident = pers.tile([P, P], F32)
            make_identity(nc, ident[:])

            # per-edge metadata as [128, 4B] (covers T used tiles)
            srcT = pers.tile([P, 4 * B], I32)
            dstT = pers.tile([P, 4 * B], F32)
            invT = pers.tile([P, 4 * B], F32)
            for (dsttile, dram) in ((srcT, d_src), (dstT, d_dst), (invT, d_inv)):
                nc.sync.dma_start(
                    out=dsttile[:],
                    in_=dram.ap().rearrange("(t p) o -> p (t o)", p=P),
                )

            # edge attrs, feature-major augmented [7, EP]; loaded once
            eaT = pers.tile([IN_W + 1, EP], F32R)
            nc.sync.dma_start(out=eaT[:].bitcast(F32), in_=d_ea[:])

            # h feature-major augmented [65, 512]; row 64 = ones
            # (hfm0 pre-rounded on host; device relu copies re-round later)
            hfm = pers.tile([WIDTH + 1, NLOC], F32R)
            nc.sync.dma_start(out=hfm[0:WIDTH, :].bitcast(F32), in_=d_hfm0[:])
            nc.vector.memset(hfm[WIDTH : WIDTH + 1, :].bitcast(F32), 1.0)

            fc2r = pers.tile([WIDTH + 1, 1], F32R)
            nc.sync.dma_start(out=fc2r[:].bitcast(F32), in_=d_fc2[:])

            # double-buffered per-layer weights
            kw1r = [pers.tile([IN_W + 1, KER_W // 2], F32R, name=f"kw1r{b}")
                    for b in range(2)]
            kw2rc = [
                [pers.tile([P, KER_W], BF16, name=f"kw2rc{b}_{c}")
                 for c in range(KER_W // 2 // P)]
                for b in range(2)
            ]
            kw3rc = [
                [pers.tile([P, NC3], BF16, name=f"kw3rc{b}_{c}") for c in range(KC3)]
                for b in range(2)
            ]
            rootr = [pers.tile([WIDTH + 1, WIDTH], F32R, name=f"rootr{b}")
                     for b in range(2)]
            kb2t = [pers.tile([P, KER_W // P], F32, name=f"kb2t{b}")
                    for b in range(2)]
            kb3t = [pers.tile([WIDTH, WIDTH], F32R, name=f"kb3t{b}")
                    for b in range(2)] if kb3_nonzero else None

            e1r = pers.tile([P, 4 * 512], BF16)
            e2r = pers.tile([P, 8 * 512], BF16)

            def load_weights(k, eng, eng2=None):
                bi = k % 2
                eng2 = eng2 or eng
                eng.dma_start(out=kw1r[bi][:].bitcast(F32), in_=d_kw1[k])
                for c in range(KER_W // 2 // P):
                    (eng if c % 2 == 0 else eng2).dma_start(
                        out=kw2rc[bi][c][:], in_=d_kw2[k, c * P : (c + 1) * P, :]
                    )
                eng.dma_start(out=rootr[bi][:].bitcast(F32), in_=d_root[k])
                eng.dma_start(out=kb2t[bi][:], in_=d_kb2[k])
                if kb3_nonzero:
                    eng.dma_start(out=kb3t[bi][:].bitcast(F32), in_=d_kb3[k])
                for kc in range(KC3):
                    (eng if kc % 2 == 0 else eng2).dma_start(
                        out=kw3rc[bi][kc][:], in_=d_kw3[k, kc * P : (kc + 1) * P, :]
                    )

            load_weights(0, nc.sync, nc.scalar)

            for k in range(DEPTH):
                bi = k % 2
                aggP = ppa.tile([WIDTH, NLOC], F32, tag="aggP")
                htab = d_h0g if k == 0 else d_hgat[k - 1]
                pend = [None]  # pending (msgr, S, t) awaiting scatter matmul

                def flush_scatter():
                    if pend[0] is None:
                        return
                    msgr_p, S_p, t_p = pend[0]
                    nc.tensor.matmul(
                        out=aggP[:], lhsT=msgr_p[:], rhs=S_p[:],
                        start=(t_p == 0), stop=False, skip_group_check=True,
                    )
                    pend[0] = None

                for blk in range(B):
                    eoff = blk * 512
                    # ---- e1 = relu(ea @ kw1_aug) : [512 feats, 512 edges] ----
                    for mc in range(4):
                        pe1 = ppe.tile([P, 512], F32, tag="pe")
                        nc.tensor.matmul(
                            out=pe1[:],
                            lhsT=kw1r[bi][:, mc * P : (mc + 1) * P],
                            rhs=eaT[:, eoff : eoff + 512],
                            start=True,
                            stop=True,
                        )
                        nc.scalar.activation(
                            e1r[:, mc * 512 : (mc + 1) * 512], pe1[:], AF.Relu
                        )
                    # prefetch next layer's weights once layer k is underway
                    if blk == 1 and k + 1 < DEPTH:
                        load_weights(k + 1, nc.gpsimd)
                    # ---- e2 = relu(e1 @ kw2 + kb2) : [1024 feats, 512 edges] ----
                    for mc2 in range(8):
                        pe2 = ppe.tile([P, 512], F32, tag="pe")
                        for kc in range(4):
                            nc.tensor.matmul(
                                out=pe2[:],
                                lhsT=kw2rc[bi][kc][:, mc2 * P : (mc2 + 1) * P],
                                rhs=e1r[:, kc * 512 : (kc + 1) * 512],
                                start=(kc == 0),
                                stop=(kc == 3),
                            )
                        nc.scalar.activation(
                            e2r[:, mc2 * 512 : (mc2 + 1) * 512],
                            pe2[:],
                            AF.Relu,
                            bias=kb2t[bi][:, mc2 : mc2 + 1],
                        )
                    # ---- per 128-edge tile (ragged last block) ----
                    for t4 in range(min(4, T - blk * 4)):
                        t = blk * 4 + t4
                        hsrc_b = wk.tile([P, WIDTH], BF16, tag="hsrcb")
                        nc.gpsimd.indirect_dma_start(
                            out=hsrc_b[:],
                            out_offset=None,
                            in_=htab[:],
                            in_offset=bass.IndirectOffsetOnAxis(
                                ap=srcT[:, t : t + 1], axis=0
                            ),
                        )
                        hsrc = wk.tile([P, WIDTH], F32, tag="hsrc")
                        nc.gpsimd.tensor_copy(out=hsrc[:], in_=hsrc_b[:])
                        accs = [
                            wk.tile([P, WIDTH], F32, name=f"macc{j}_{t}",
                                    tag=f"macc{j}", bufs=2)
                            for j in range(NACCS)
                        ]
                        msgr = wk.tile([P, WIDTH], F32R, tag="msgr")
                        tcor = None
                        if kb3_nonzero:
                            tps = ppm.tile([WIDTH, P], BF16, tag="tp2")
                            nc.tensor.transpose(out=tps[:], in_=hsrc_b[:], identity=ident[:])
                            hsT = wk.tile([WIDTH, P], F32R, tag="hsT")
                            nc.scalar.activation(hsT[:], tps[:], AF.Copy)
                            tcor = ppm.tile([P, WIDTH], F32, tag="tc")
                            nc.tensor.matmul(
                                out=tcor[:], lhsT=hsT[:], rhs=kb3t[bi][:],
                                start=True, stop=True,
                            )
                        for cc in range(8):  # 512-col chunks of We
                            wps = ppw.tile([P, 512], F32, tag="wps")
                            for kc in range(KC3):
                                nc.tensor.matmul(
                                    out=wps[:],
                                    lhsT=e2r[:, kc * 512 + t4 * P : kc * 512 + (t4 + 1) * P],
                                    rhs=kw3rc[bi][kc][:, cc * 512 : (cc + 1) * 512],
                                    start=(kc == 0),
                                    stop=(kc == KC3 - 1),
                                )
                            # stage PSUM->SBUF on Act: DVE reads from SBUF are
                            # ~2x cheaper than PSUM reads
                            wsb = wk.tile([P, 512], F32, tag="wsb")
                            nc.scalar.activation(wsb[:], wps[:], AF.Copy)
                            if cc == 0:
                                # scatter for the previous tile: emitted after
                                # this tile's first We chunk so the PE never
                                # waits on the DVE matvec tail
                                flush_scatter()
                            for j in range(8):
                                i_ = cc * 8 + j
                                sl = wsb[:, j * WIDTH : (j + 1) * WIDTH]
                                sc = hsrc[:, i_ : i_ + 1]
                                ja = i_ % NACCS
                                if i_ < NACCS:
                                    nc.vector.tensor_scalar(
                                        out=accs[ja][:], in0=sl, scalar1=sc,
                                        scalar2=None, op0=ALU.mult,
                                    )
                                else:
                                    nc.vector.scalar_tensor_tensor(
                                        out=accs[ja][:], in0=sl, scalar=sc,
                                        in1=accs[ja][:], op0=ALU.mult, op1=ALU.add,
                                    )
                        # tree-reduce the 8 accumulators
                        for d in (0, 2, 4, 6):
                            nc.vector.tensor_add(
                                out=accs[d][:], in0=accs[d][:], in1=accs[d + 1][:]
                            )
                        nc.vector.tensor_add(out=accs[0][:], in0=accs[0][:], in1=accs[2][:])
                        if kb3_nonzero:
                            nc.vector.tensor_add(out=accs[4][:], in0=accs[4][:], in1=accs[6][:])
                            nc.vector.tensor_add(out=accs[0][:], in0=accs[0][:], in1=accs[4][:])
                            nc.vector.tensor_add(out=msgr[:], in0=accs[0][:], in1=tcor[:])
                        else:
                            nc.vector.tensor_add(out=accs[4][:], in0=accs[4][:], in1=accs[6][:])
                            nc.vector.tensor_add(out=msgr[:], in0=accs[0][:], in1=accs[4][:])
                        # ---- one-hot scatter weights (iota==dst)*invdeg ----
                        S = wk.tile([P, NLOC], F32R, tag="S")
                        nc.vector.tensor_scalar(
                            out=S[:], in0=iota_s[:], scalar1=dstT[:, t : t + 1],
                            scalar2=invT[:, t : t + 1], op0=ALU.is_equal, op1=ALU.mult,
                        )
                        pend[0] = (msgr, S, t)
                # ---- update: h = relu(agg*inv_deg(folded) + h@root + bias) ----
                flush_scatter()
                nc.tensor.matmul(
                    out=aggP[:], lhsT=rootr[bi][:], rhs=hfm[:],
                    start=False, stop=True, skip_group_check=True,
                )
                hnf = wk.tile([WIDTH, NLOC], F32, tag="hnf")
                nc.scalar.activation(hnf[:], aggP[:], AF.Relu)
                nc.scalar.activation(hfm[0:WIDTH, :], hnf[:], AF.Copy)
                if k < DEPTH - 1:
                    for c in range(NLOC // P):
                        tp = ppm.tile([P, WIDTH], F32, tag="tp")
                        nc.tensor.transpose(
                            out=tp[:],
                            in_=hnf[:, c * P : (c + 1) * P],
                            identity=ident[0:WIDTH, 0:WIDTH],
                        )
                        hts = wk.tile([P, WIDTH], BF16, tag="hts")
                        nc.vector.tensor_copy(out=hts[:], in_=tp[:])
                        nc.sync.dma_start(out=d_hloc[c * P : (c + 1) * P, :], in_=hts[:])
                    nc.gpsimd.collective_compute(
                        "AllGather",
                        ALU.bypass,
                        ins=[d_hloc[:]],
                        outs=[d_hgat[k][:]],
                        replica_groups=rg,
                    )
            # ---- readout: out = h @ fc2 + b ----
            pf = ppm.tile([1, NLOC], F32, tag="tp")
            nc.tensor.matmul(out=pf[:], lhsT=fc2r[:], rhs=hfm[:], start=True, stop=True)
            ot = wk.tile([1, NLOC], F32, tag="hnf")
            nc.vector.tensor_copy(out=ot[:], in_=pf[:])
            nc.sync.dma_start(out=d_out[:], in_=ot[:])

    _split_excess_waits(nc)
    return nc


def _host_prep(x, edge_attr, fc1_w, fc1_b, kw1, kb1, kw2, kb2, kw3, kb3,
               root, bias, fc2_w, fc2_b, edge_index):
    f = np.float32
    bf = ml_dtypes.bfloat16
    x = np.asarray(x, f)
    edge_attr = np.asarray(edge_attr, f)
    fc1_w = np.asarray(fc1_w, f); fc1_b = np.asarray(fc1_b, f)
    kw1 = np.asarray(kw1, f); kb1 = np.asarray(kb1, f)
    kw2 = np.asarray(kw2, f); kb2 = np.asarray(kb2, f)
    kw3 = np.asarray(kw3, f); kb3 = np.asarray(kb3, f)
    root = np.asarray(root, f); bias = np.asarray(bias, f)
    fc2_w = np.asarray(fc2_w, f); fc2_b = np.asarray(fc2_b, f)
    ei = np.asarray(edge_index)
    src = ei[0].astype(np.int64)
    dst = ei[1].astype(np.int64)

    deg = np.bincount(dst, minlength=N_NODES).astype(f)
    inv_deg = np.zeros(N_NODES, f)
    np.divide(f(1.0), deg, out=inv_deg, where=deg > 0)

    order = np.argsort(dst, kind="stable")
    dsts = dst[order]
    bounds = np.searchsorted(dsts, np.arange(0, N_NODES + 1, NLOC))
    counts = np.diff(bounds)
    T = int(np.ceil(counts.max() / 128.0))
    EP = ((T + 3) // 4) * 512

    h0 = (x @ fc1_w + fc1_b).astype(f)

    kw1_aug = _round_f32r(np.concatenate([kw1, kb1[:, None, :]], axis=1))
    kw2b = kw2.astype(bf)
    kw3b = kw3.astype(bf)
    kb2s = np.stack([kb2[k].reshape(KER_W // P, P).T for k in range(DEPTH)]).astype(f)
    root_aug = _round_f32r(np.concatenate([root, bias[:, None, :]], axis=1))
    fc2_aug = _round_f32r(np.concatenate([fc2_w, fc2_b.reshape(1, 1)], axis=0))
    iota = np.tile(np.arange(NLOC, dtype=f)[None, :], (P, 1))
    kb3_nonzero = bool(np.any(kb3))
    kb3r = _round_f32r(kb3.reshape(DEPTH, WIDTH, WIDTH))

    in_maps = []
    for m in range(NCORES):
        sel = order[bounds[m] : bounds[m + 1]]
        cnt = len(sel)
        eaT = np.zeros((IN_W + 1, EP), f)
        eaT[0:IN_W, :cnt] = edge_attr[sel].T
        eaT[IN_W, :cnt] = 1.0
        eaT = _round_f32r(eaT)
        srci = np.zeros((EP, 1), np.int32)
        srci[:cnt, 0] = src[sel].astype(np.int32)
        dstl = np.full((EP, 1), -1.0, f)
        dstl[:cnt, 0] = (dst[sel] - NLOC * m).astype(f)
        invde = np.zeros((EP, 1), f)
        invde[:cnt, 0] = inv_deg[dst[sel]]
        im = {
            "h0g": h0.astype(bf),
            "hfm0": _round_f32r(np.ascontiguousarray(h0[NLOC * m : NLOC * (m + 1)].T)),
            "kw1a": kw1_aug,
            "kw2b": kw2b,
            "kb2s": kb2s,
            "kw3b": kw3b,
            "roota": root_aug,
            "fc2a": fc2_aug,
            "eaT": eaT,
            "srci": srci,
            "dstl": dstl,
            "invde": invde,
            "iota": iota,
        }
        if kb3_nonzero:
            im["kb3r"] = kb3r
        in_maps.append(im)
    return in_maps, T, kb3_nonzero


_BUILD_CACHE = {}


def kernel(**inputs) -> np.ndarray:
    in_maps, T, kb3_nonzero = _host_prep(**inputs)
    key = (T, kb3_nonzero)
    if key not in _BUILD_CACHE:
        _BUILD_CACHE[key] = _build_nc(T, kb3_nonzero)
    nc = _BUILD_CACHE[key]
    res = run_bass_kernel_spmd(nc, in_maps, list(range(NCORES)))
    out = np.concatenate(
        [res.results[m]["out_loc"].reshape(NLOC, 1) for m in range(NCORES)], axis=0
    )
    return out.astype(np.float32)


# revision 11
# speedup vs baseline: 1.0355x; 1.0160x over previous
"""Trainium2 Bass kernel for DeepKernelNN GNN message passing (NNConv-style).

Strategy (8 NeuronCores, SPMD):
  - Host: sort edges by dst, shard contiguous 512-node dst ranges per core,
    pad each core to a common edge count. Precompute h0 = x@fc1+b (tiny),
    per-edge metadata (src idx, local dst, 1/deg), and augmented weights.
  - Device per layer: edge MLP feature-major (weights stationary on PE),
    We = e2@kw3 edge-major in bf16 into PSUM, staged to SBUF by the Act
    engine, per-edge matvec msg = h[src] . We on DVE with 8 independent
    per-partition-scalar FMA accumulators, segment-sum via one-hot scatter
    matmul (S built on device from iota/is_equal, inv_deg folded in; the
    scatter matmul is software-pipelined one tile behind the We matmuls so
    the PE never waits on the DVE), NNConv update feature-major.
  - The update + AllGather are split into node halves: the low half (local
    nodes 0..255) finalizes mid-layer once its dst-sorted edge tiles are
    done, so its AllGather overlaps the rest of the layer; only the high
    half's collective is exposed at the layer boundary.  The gather table
    uses a split layout (rank-major halves) to match; h tables are bf16.
  - The e1 stage of block b+1 and the e1+e2 stages of the next layer's
    first block are emitted early (software pipelining) so the PE has work
    during Act/collective latency.  Layer k+1 weights (kw2/kw3 in bf16)
    are double-buffered and prefetched on the gpsimd DMA queue.
"""

import sys

sys.path.insert(0, "/opt/trn_rl_repo")

import numpy as np
import ml_dtypes

import concourse.bass as bass
import concourse.mybir as mybir
import concourse.tile as tile_mod
from concourse.bass_utils import run_bass_kernel_spmd
from concourse.masks import make_identity
from concourse.tile import TileContext
from concourse.vector_clock import ScopedClock, VectorClock

# ----------------------------------------------------------------------------
# Toolchain workarounds: this walrus build rejects instructions carrying more
# than a couple of sync waits ("Too many sync wait commands").  Split waits
# onto dedicated same-engine NoOps.
# ----------------------------------------------------------------------------
WAIT_LIMIT = 1


def _patched_drain_and_barrier(self, tick_clock, wait_clock):
    nc = self.nc
    gc = tick_clock.global_clock
    n = len(gc)
    for i in range(n):
        t = gc[i]
        if t > 0:
            sub = [0] * n
            sub[i] = t
            nop_inst = nc.sync.nop(nofuse=True)
            wait_clock.add_sem_waits(nop_inst.ins, ScopedClock({None: VectorClock(sub)}))
    nc.sync.drain()
    nc.all_engine_barrier()
    popped = nc._tile_sem_poison_stack.pop()
    assert popped is self._sem_poison
    nc.clear_and_free_semaphores(list(self.sems.allocated().values()))
    nc.all_engine_barrier()


tile_mod.TileContext._drain_and_barrier = _patched_drain_and_barrier


def _split_excess_waits(nc, limit=WAIT_LIMIT):
    n_split = 0
    for _bbname, bbb in nc.bb_map.items():
        bb = bbb.bb
        insts = list(bb.instructions)
        out = []
        for inst in insts:
            si = inst.sync_info
            if si is not None and si.on_wait is not None and len(si.on_wait) > limit:
                waits = list(si.on_wait)
                movable = [w for w in waits if w.wait_reg is None]
                fixed = [w for w in waits if w.wait_reg is not None]
                nkeep_mov = max(0, limit - len(fixed))
                keep = fixed + (movable[len(movable) - nkeep_mov:] if nkeep_mov else [])
                extra = movable[: len(movable) - nkeep_mov]
                while extra:
                    chunk, extra = extra[:limit], extra[limit:]
                    nop = mybir.InstNoOp(name=nc.get_next_instruction_name())
                    nop.engine = inst.engine
                    nop.sync_info = mybir.SyncInfo(on_wait=chunk, on_update=[])
                    nop.bass_nofuse = True
                    nc.register_instruction(nop, overwrite=True)
                    out.append(nop)
                    n_split += 1
                inst.sync_info = mybir.SyncInfo(
                    on_wait=keep, on_update=list(si.on_update or [])
                )
            out.append(inst)
        bb.instructions = out
    return n_split


import concourse.bass_utils as _bu

_orig_run_command = _bu.run_command


def _run_command_no_birverifier(cmd, **kw):
    cmd = [
        c.replace("birverifier,", "") if isinstance(c, str) else c for c in cmd
    ]
    return _orig_run_command(cmd, **kw)


_bu.run_command = _run_command_no_birverifier


def _round_f32r(x):
    """Host replica of the fp32r rounding (RNE, 11 mantissa bits kept).

    PE fp32r operands must contain rounded bits: feeding raw fp32 bits makes
    the PE fault (verified on HW), so anything DMA'd into an fp32r tile is
    pre-rounded here.
    """
    ai = np.ascontiguousarray(x, np.float32).view(np.uint32)
    drop = np.uint32(12)
    half = np.uint32(1 << 11)
    lsb = ((ai >> drop) & np.uint32(1)).astype(np.uint32)
    out = ((ai + (half - np.uint32(1)) + lsb) >> drop) << drop
    return out.view(np.float32)


# ----------------------------------------------------------------------------
# Problem constants (hardcoded from the model definition)
# ----------------------------------------------------------------------------
N_NODES = 4096
N_EDGES = 32768
WIDTH = 64
KER_W = 1024
DEPTH = 4
KER_IN = 6
IN_W = 6
NCORES = 8
NLOC = N_NODES // NCORES  # 512 nodes per core
NHALF = NLOC // 2
P = 128

_dt = mybir.dt
F32 = _dt.float32
F32R = _dt.float32r
BF16 = _dt.bfloat16
I32 = _dt.int32
ALU = mybir.AluOpType
AF = mybir.ActivationFunctionType
NACCS = 8
KC3 = KER_W // P  # 8 contraction chunks for kw3
NC3 = WIDTH * WIDTH  # 4096 output cols


def _build_nc(T, kb3_nonzero, t_half, t_hi):
    """Build the SPMD Bass program for T 128-edge tiles per core.

    t_half: last tile index (global max over cores) containing an edge with
            local dst < NHALF -- the low-half update runs after it.
    t_hi:   first tile index (global min) containing an edge with local
            dst >= NHALF.
    """
    B = (T + 3) // 4  # blocks of 512 edges (last may be ragged)
    EP = B * 512
    nc = bass.Bass("TRN2", target_bir_lowering=False, debug=False, num_devices=NCORES)

    # ---- dram inputs (per-core in_maps supply the data) ----
    d_h0g = nc.dram_tensor("h0g", [N_NODES, WIDTH], BF16, kind="ExternalInput")
    d_hfm0 = nc.dram_tensor("hfm0", [WIDTH, NLOC], F32, kind="ExternalInput")
    d_kw1 = nc.dram_tensor("kw1a", [DEPTH, IN_W + 1, KER_W // 2], F32, kind="ExternalInput")
    d_kw2 = nc.dram_tensor("kw2b", [DEPTH, KER_W // 2, KER_W], BF16, kind="ExternalInput")
    d_kb2 = nc.dram_tensor("kb2s", [DEPTH, P, KER_W // P], F32, kind="ExternalInput")
    d_kw3 = nc.dram_tensor("kw3b", [DEPTH, KER_W, WIDTH * WIDTH], BF16, kind="ExternalInput")
    d_root = nc.dram_tensor("roota", [DEPTH, WIDTH + 1, WIDTH], F32, kind="ExternalInput")
    d_fc2 = nc.dram_tensor("fc2a", [WIDTH + 1, 1], F32, kind="ExternalInput")
    d_ea = nc.dram_tensor("eaT", [IN_W + 1, EP], F32, kind="ExternalInput")
    d_src = nc.dram_tensor("srci", [EP, 1], I32, kind="ExternalInput")
    d_dst = nc.dram_tensor("dstl", [EP, 1], F32, kind="ExternalInput")
    d_inv = nc.dram_tensor("invde", [EP, 1], F32, kind="ExternalInput")
    d_iota = nc.dram_tensor("iota", [P, NLOC], F32, kind="ExternalInput")
    d_kb3 = None
    if kb3_nonzero:
        d_kb3 = nc.dram_tensor("kb3r", [DEPTH, WIDTH, WIDTH], F32, kind="ExternalInput")

    d_out = nc.dram_tensor("out_loc", [1, NLOC], F32, kind="ExternalOutput")

    # ---- internal dram ----
    d_hloc = nc.dram_tensor("hloc", [NLOC, WIDTH], BF16)
    d_hgat = [
        nc.dram_tensor(f"hgat{k}", [N_NODES, WIDTH], BF16, addr_space="Shared")
        for k in range(DEPTH - 1)
    ]

    rg = [list(range(NCORES))]

    with TileContext(nc) as tc:
        with (
            tc.tile_pool(name="pers", bufs=1) as pers,
            tc.tile_pool(name="wk", bufs=2) as wk,
            tc.tile_pool(name="ppw", bufs=2, space="PSUM") as ppw,
            tc.tile_pool(name="ppe", bufs=2, space="PSUM") as ppe,
            tc.tile_pool(name="ppm", bufs=2, space="PSUM") as ppm,
            tc.tile_pool(name="ppa", bufs=1, space="PSUM") as ppa,
        ):
            # ---------------- persistent tiles ----------------
            # (allocation order != DMA issue order; critical loads first)
            iota_s = pers.tile([P, NLOC], F32)
            ident = pers.tile([P, P], F32)
            srcT = pers.tile([P, 4 * B], I32)
            dstT = pers.tile([P, 4 * B], F32)
            invT = pers.tile([P, 4 * B], F32)
            eaT = pers.tile([IN_W + 1, EP], F32R)
            hfm = pers.tile([WIDTH + 1, NLOC], F32R)
            fc2r = pers.tile([WIDTH + 1, 1], F32R)
            kw1r = [pers.tile([IN_W + 1, KER_W // 2], F32R, name=f"kw1r{b}")
                    for b in range(2)]
            kw2rc = [
                [pers.tile([P, KER_W], BF16, name=f"kw2rc{b}_{c}")
                 for c in range(KER_W // 2 // P)]
                for b in range(2)
            ]
            kw3rc = [
                [pers.tile([P, NC3], BF16, name=f"kw3rc{b}_{c}") for c in range(KC3)]
                for b in range(2)
            ]
            rootr = [pers.tile([WIDTH + 1, WIDTH], F32R, name=f"rootr{b}")
                     for b in range(2)]
            kb2t = [pers.tile([P, KER_W // P], F32, name=f"kb2t{b}")
                    for b in range(2)]
            kb3t = [pers.tile([WIDTH, WIDTH], F32R, name=f"kb3t{b}")
                    for b in range(2)] if kb3_nonzero else None
            e1r = pers.tile([P, 4 * 512], BF16)
            e2r = pers.tile([P, 8 * 512], BF16)

            def load_weights(k, engs):
                bi = k % 2
                engs[0].dma_start(out=kw1r[bi][:].bitcast(F32), in_=d_kw1[k])
                for c in range(KER_W // 2 // P):
                    engs[c % len(engs)].dma_start(
                        out=kw2rc[bi][c][:], in_=d_kw2[k, c * P : (c + 1) * P, :]
                    )
                for kc in range(KC3):
                    engs[kc % len(engs)].dma_start(
                        out=kw3rc[bi][kc][:], in_=d_kw3[k, kc * P : (kc + 1) * P, :]
                    )
                engs[0].dma_start(out=rootr[bi][:].bitcast(F32), in_=d_root[k])
                engs[0].dma_start(out=kb2t[bi][:], in_=d_kb2[k])
                if kb3_nonzero:
                    engs[0].dma_start(out=kb3t[bi][:].bitcast(F32), in_=d_kb3[k])

            # critical-path loads first: block-0 inputs + layer-0 weights,
            # spread over three DMA queues; bulk/late-need tensors after
            nc.sync.dma_start(
                out=srcT[:], in_=d_src.ap().rearrange("(t p) o -> p (t o)", p=P)
            )
            nc.scalar.dma_start(out=eaT[:, 0:512].bitcast(F32), in_=d_ea[:, 0:512])
            load_weights(0, [nc.sync, nc.scalar, nc.gpsimd])
            nc.sync.dma_start(out=iota_s[:], in_=d_iota[:])
            make_identity(nc, ident[:])
            for (dsttile, dram) in ((dstT, d_dst), (invT, d_inv)):
                nc.sync.dma_start(
                    out=dsttile[:], in_=dram.ap().rearrange("(t p) o -> p (t o)", p=P)
                )
            nc.scalar.dma_start(
                out=eaT[:, 512:EP].bitcast(F32), in_=d_ea[:, 512:EP]
            )
            nc.sync.dma_start(out=hfm[0:WIDTH, :].bitcast(F32), in_=d_hfm0[:])
            nc.vector.memset(hfm[WIDTH : WIDTH + 1, :].bitcast(F32), 1.0)
            nc.sync.dma_start(out=fc2r[:].bitcast(F32), in_=d_fc2[:])

            def emit_e1(k, blk):
                bi = k % 2
                eoff = blk * 512
                for mc in range(4):
                    pe1 = ppe.tile([P, 512], F32, tag="pe")
                    nc.tensor.matmul(
                        out=pe1[:],
                        lhsT=kw1r[bi][:, mc * P : (mc + 1) * P],
                        rhs=eaT[:, eoff : eoff + 512],
                        start=True,
                        stop=True,
                    )
                    nc.scalar.activation(
                        e1r[:, mc * 512 : (mc + 1) * 512], pe1[:], AF.Relu
                    )

            def emit_e2(k, blk):
                bi = k % 2
                for mc2 in range(8):
                    pe2 = ppe.tile([P, 512], F32, tag="pe")
                    for kc in range(4):
                        nc.tensor.matmul(
                            out=pe2[:],
                            lhsT=kw2rc[bi][kc][:, mc2 * P : (mc2 + 1) * P],
                            rhs=e1r[:, kc * 512 : (kc + 1) * 512],
                            start=(kc == 0),
                            stop=(kc == 3),
                        )
                    nc.scalar.activation(
                        e2r[:, mc2 * 512 : (mc2 + 1) * 512],
                        pe2[:],
                        AF.Relu,
                        bias=kb2t[bi][:, mc2 : mc2 + 1],
                    )

            emit_e1(0, 0)
            emit_e2(0, 0)

            for k in range(DEPTH):
                bi = k % 2
                aggP = ppa.tile([WIDTH, NLOC], F32, tag="aggP")
                htab = d_h0g if k == 0 else d_hgat[k - 1]
                pend = [None]  # pending (msgr, S, t) awaiting scatter matmul

                def flush_scatter():
                    if pend[0] is None:
                        return None
                    msgr_p, S_p, t_p = pend[0]
                    nc.tensor.matmul(
                        out=aggP[:], lhsT=msgr_p[:], rhs=S_p[:],
                        start=(t_p == 0), stop=False, skip_group_check=True,
                    )
                    pend[0] = None
                    return t_p

                def emit_update_full():
                    nc.tensor.matmul(
                        out=aggP[:], lhsT=rootr[bi][:], rhs=hfm[:],
                        start=False, stop=True, skip_group_check=True,
                    )
                    hnf = wk.tile([WIDTH, NLOC], F32, tag="hnfl")
                    nc.scalar.activation(hnf[:], aggP[:], AF.Relu)
                    nc.scalar.activation(hfm[0:WIDTH, :], hnf[:], AF.Copy)
                    if k < DEPTH - 1:
                        for c in range(NLOC // P):
                            tp = ppm.tile([P, WIDTH], F32, tag="tp")
                            nc.tensor.transpose(
                                out=tp[:],
                                in_=hnf[:, c * P : (c + 1) * P],
                                identity=ident[0:WIDTH, 0:WIDTH],
                            )
                            hts = wk.tile([P, WIDTH], BF16, tag="hts")
                            nc.vector.tensor_copy(out=hts[:], in_=tp[:])
                            nc.sync.dma_start(
                                out=d_hloc[c * P : (c + 1) * P, :], in_=hts[:]
                            )
                        nc.gpsimd.collective_compute(
                            "AllGather",
                            ALU.bypass,
                            ins=[d_hloc[:]],
                            outs=[d_hgat[k][:]],
                            replica_groups=rg,
                        )

                for blk in range(B):
                    if blk == 1 and k + 1 < DEPTH:
                        load_weights(k + 1, [nc.gpsimd])
                    if blk > 0:
                        emit_e2(k, blk)
                    # ---- per 128-edge tile (ragged last block) ----
                    for t4 in range(min(4, T - blk * 4)):
                        t = blk * 4 + t4
                        hsrc_b = wk.tile([P, WIDTH], BF16, tag="hsrcb")
                        nc.gpsimd.indirect_dma_start(
                            out=hsrc_b[:],
                            out_offset=None,
                            in_=htab[:],
                            in_offset=bass.IndirectOffsetOnAxis(
                                ap=srcT[:, t : t + 1], axis=0
                            ),
                        )
                        hsrc = wk.tile([P, WIDTH], F32, tag="hsrc")
                        nc.gpsimd.tensor_copy(out=hsrc[:], in_=hsrc_b[:])
                        accs = [
                            wk.tile([P, WIDTH], F32, name=f"macc{j}_{t}",
                                    tag=f"macc{j}", bufs=1)
                            for j in range(NACCS)
                        ]
                        msgr = wk.tile([P, WIDTH], F32R, tag="msgr")
                        tcor = None
                        if kb3_nonzero:
                            tps = ppm.tile([WIDTH, P], BF16, tag="tp2")
                            nc.tensor.transpose(out=tps[:], in_=hsrc_b[:], identity=ident[:])
                            hsT = wk.tile([WIDTH, P], F32R, tag="hsT")
                            nc.scalar.activation(hsT[:], tps[:], AF.Copy)
                            tcor = ppm.tile([P, WIDTH], F32, tag="tc")
                            nc.tensor.matmul(
                                out=tcor[:], lhsT=hsT[:], rhs=kb3t[bi][:],
                                start=True, stop=True,
                            )
                        flushed = [None]
                        for cc in range(8):  # 512-col chunks of We
                            wps = ppw.tile([P, 512], F32, tag="wps")
                            for kc in range(KC3):
                                nc.tensor.matmul(
                                    out=wps[:],
                                    lhsT=e2r[:, kc * 512 + t4 * P : kc * 512 + (t4 + 1) * P],
                                    rhs=kw3rc[bi][kc][:, cc * 512 : (cc + 1) * 512],
                                    start=(kc == 0),
                                    stop=(kc == KC3 - 1),
                                )
                            # stage PSUM->SBUF on Act: DVE reads from SBUF are
                            # ~2x cheaper than PSUM reads
                            wsb = wk.tile([P, 512], BF16, tag="wsb", bufs=8)
                            nc.scalar.activation(wsb[:], wps[:], AF.Copy)
                            if cc == 0:
                                # scatter for the previous tile: emitted after
                                # this tile's first We chunk so the PE never
                                # waits on the DVE matvec tail
                                flushed[0] = flush_scatter()
                            for j in range(8):
                                i_ = cc * 8 + j
                                sl = wsb[:, j * WIDTH : (j + 1) * WIDTH]
                                sc = hsrc[:, i_ : i_ + 1]
                                ja = i_ % NACCS
                                if i_ < NACCS:
                                    nc.vector.tensor_scalar(
                                        out=accs[ja][:], in0=sl, scalar1=sc,
                                        scalar2=None, op0=ALU.mult,
                                    )
                                else:
                                    nc.vector.scalar_tensor_tensor(
                                        out=accs[ja][:], in0=sl, scalar=sc,
                                        in1=accs[ja][:], op0=ALU.mult, op1=ALU.add,
                                    )
                        # tree-reduce the 8 accumulators
                        for d in (0, 2, 4, 6):
                            nc.vector.tensor_add(
                                out=accs[d][:], in0=accs[d][:], in1=accs[d + 1][:]
                            )
                        nc.vector.tensor_add(out=accs[0][:], in0=accs[0][:], in1=accs[2][:])
                        if kb3_nonzero:
                            nc.vector.tensor_add(out=accs[4][:], in0=accs[4][:], in1=accs[6][:])
                            nc.vector.tensor_add(out=accs[0][:], in0=accs[0][:], in1=accs[4][:])
                            nc.vector.tensor_add(out=msgr[:], in0=accs[0][:], in1=tcor[:])
                        else:
                            nc.vector.tensor_add(out=accs[4][:], in0=accs[4][:], in1=accs[6][:])
                            nc.vector.tensor_add(out=msgr[:], in0=accs[0][:], in1=accs[4][:])
                        # ---- one-hot scatter weights (iota==dst)*invdeg ----
                        S = wk.tile([P, NLOC], F32R, tag="S")
                        nc.vector.tensor_scalar(
                            out=S[:], in0=iota_s[:], scalar1=dstT[:, t : t + 1],
                            scalar2=invT[:, t : t + 1], op0=ALU.is_equal, op1=ALU.mult,
                        )
                        pend[0] = (msgr, S, t)
                        # software-pipeline the next block's e1 stage
                        if t4 == 2 and blk + 1 < B:
                            emit_e1(k, blk + 1)
                # ---- layer tail: flush, high-half update, next-layer prework ----
                flush_scatter()
                emit_update_full()
                if k + 1 < DEPTH:
                    emit_e1(k + 1, 0)
                    emit_e2(k + 1, 0)
            # ---- readout: out = h @ fc2 + b ----
            pf = ppm.tile([1, NLOC], F32, tag="tp")
            nc.tensor.matmul(out=pf[:], lhsT=fc2r[:], rhs=hfm[:], start=True, stop=True)
            ot = wk.tile([1, NLOC], F32, tag="ot")
            nc.vector.tensor_copy(out=ot[:], in_=pf[:])
            nc.sync.dma_start(out=d_out[:], in_=ot[:])

    _split_excess_waits(nc)
    return nc


def _split_remap(n):
    """Row index of global node n in the split-layout gather table."""
    q, r = np.divmod(n, NLOC)
    lo = r < NHALF
    return np.where(lo, q * NHALF + r, N_NODES // 2 + q * NHALF + (r - NHALF))


def _host_prep(x, edge_attr, fc1_w, fc1_b, kw1, kb1, kw2, kb2, kw3, kb3,
               root, bias, fc2_w, fc2_b, edge_index):
    f = np.float32
    bf = ml_dtypes.bfloat16
    x = np.asarray(x, f)
    edge_attr = np.asarray(edge_attr, f)
    fc1_w = np.asarray(fc1_w, f); fc1_b = np.asarray(fc1_b, f)
    kw1 = np.asarray(kw1, f); kb1 = np.asarray(kb1, f)
    kw2 = np.asarray(kw2, f); kb2 = np.asarray(kb2, f)
    kw3 = np.asarray(kw3, f); kb3 = np.asarray(kb3, f)
    root = np.asarray(root, f); bias = np.asarray(bias, f)
    fc2_w = np.asarray(fc2_w, f); fc2_b = np.asarray(fc2_b, f)
    ei = np.asarray(edge_index)
    src = ei[0].astype(np.int64)
    dst = ei[1].astype(np.int64)

    deg = np.bincount(dst, minlength=N_NODES).astype(f)
    inv_deg = np.zeros(N_NODES, f)
    np.divide(f(1.0), deg, out=inv_deg, where=deg > 0)

    order = np.argsort(dst, kind="stable")
    dsts = dst[order]
    bounds = np.searchsorted(dsts, np.arange(0, N_NODES + 1, NLOC))
    counts = np.diff(bounds)
    T = int(np.ceil(counts.max() / 128.0))
    EP = ((T + 3) // 4) * 512

    # split-update tile thresholds (see _build_nc docstring)
    mid = np.searchsorted(dsts, np.arange(NHALF, N_NODES + 1, NLOC))[: NCORES]
    n_lo = mid - bounds[:-1]
    t_half = int(((n_lo - 1) // 128).max())
    t_hi = int((n_lo // 128).min())
    assert 0 <= t_half < T and 0 <= t_hi < T

    h0 = (x @ fc1_w + fc1_b).astype(f)

    kw1_aug = _round_f32r(np.concatenate([kw1, kb1[:, None, :]], axis=1))
    kw2b = kw2.astype(bf)
    kw3b = kw3.astype(bf)
    kb2s = np.stack([kb2[k].reshape(KER_W // P, P).T for k in range(DEPTH)]).astype(f)
    root_aug = _round_f32r(np.concatenate([root, bias[:, None, :]], axis=1))
    fc2_aug = _round_f32r(np.concatenate([fc2_w, fc2_b.reshape(1, 1)], axis=0))
    iota = np.tile(np.arange(NLOC, dtype=f)[None, :], (P, 1))
    kb3_nonzero = bool(np.any(kb3))
    kb3r = _round_f32r(kb3.reshape(DEPTH, WIDTH, WIDTH))

    in_maps = []
    for m in range(NCORES):
        sel = order[bounds[m] : bounds[m + 1]]
        cnt = len(sel)
        eaT = np.zeros((IN_W + 1, EP), f)
        eaT[0:IN_W, :cnt] = edge_attr[sel].T
        eaT[IN_W, :cnt] = 1.0
        eaT = _round_f32r(eaT)
        srci = np.zeros((EP, 1), np.int32)
        srci[:cnt, 0] = src[sel].astype(np.int32)
        dstl = np.full((EP, 1), -1.0, f)
        dstl[:cnt, 0] = (dst[sel] - NLOC * m).astype(f)
        invde = np.zeros((EP, 1), f)
        invde[:cnt, 0] = inv_deg[dst[sel]]
        im = {
            "h0g": h0.astype(bf),
            "hfm0": _round_f32r(np.ascontiguousarray(h0[NLOC * m : NLOC * (m + 1)].T)),
            "kw1a": kw1_aug,
            "kw2b": kw2b,
            "kb2s": kb2s,
            "kw3b": kw3b,
            "roota": root_aug,
            "fc2a": fc2_aug,
            "eaT": eaT,
            "srci": srci,
            "dstl": dstl,
            "invde": invde,
            "iota": iota,
        }
        if kb3_nonzero:
            im["kb3r"] = kb3r
        in_maps.append(im)
    return in_maps, T, kb3_nonzero, t_half, t_hi


_BUILD_CACHE = {}


def kernel(**inputs) -> np.ndarray:
    in_maps, T, kb3_nonzero, t_half, t_hi = _host_prep(**inputs)
    key = (T, kb3_nonzero, t_half, t_hi)
    if key not in _BUILD_CACHE:
        _BUILD_CACHE[key] = _build_nc(T, kb3_nonzero, t_half, t_hi)
    nc = _BUILD_CACHE[key]
    res = run_bass_kernel_spmd(nc, in_maps, list(range(NCORES)))
    out = np.concatenate(
        [res.results[m]["out_loc"].reshape(NLOC, 1) for m in range(NCORES)], axis=0
    )
    return out.astype(np.float32)
